# revision 48
# baseline (speedup 1.0000x reference)
"""GAT (3-layer DGL-style) on 8 Trainium2 NeuronCores.

Nodes are globally sorted by in-degree and dealt round-robin to cores/slots
so every core's dst-block degree profile matches, which minimizes the
program-wide slot-grid width (the dominant cost is SWDGE descriptor
generation at ~8 ns/gathered row on the Pool Q7, so padded slots are paid
in wall-clock). Per layer: a bf16 dense matmul produces per-node rows
[h | el | er] with h feature-interleaved (f' = hd*H + h); the rows stream
into per-group local tables whose AllGathers fire as each block group
completes, overlapping the collective with the compute pipeline; then each
core runs the edge phase for its own dst blocks. Gathers use a single pass
over a table base centered at CBASE with *signed* int16 indices (the gather
ucode computes base + idx*stride with sign-extended indices; only trailing
negative indices are dropped, so each chunk's final slot is kept
nonnegative). Per chunk: alpha = exp(lrelu(el+er)) via 2 DVE ops + 1 ACT
exp, the alpha-weighted messages via one broadcast tensor_tensor (legal
because of the feature interleave), and the slot reduction + softmax
denominator on the tensor engine as identity-weight matmuls accumulating in
PSUM. Next layer's dense blocks interleave into the edge loop. Padding
slots point at sentinel table rows (h=0, el=-200) so no masking is needed.
"""

import os

import numpy as np
import ml_dtypes

import concourse.bacc as bacc
import concourse.bass as bass
import concourse.mybir as mybir
from concourse import tile
from concourse.bass_utils import run_bass_kernel_spmd
from bass_rust import SemaphoreHandle

N = 50000
E = 800000
NC = 8
L = 6250                 # real nodes per core
NBLK = 49
LP = NBLK * 128          # padded nodes per core (6272; rows 6250+ are sentinels)
NP = NC * LP             # padded global node count (50176)
CBASE = NP // 2          # gather base row: idx are signed rel ids in [-CBASE, CBASE)

# AllGather groups: block ranges whose tables gather independently so the
# collectives overlap the dense/edge pipeline; last group smallest so the
# final exposed AG is short. Table rows are laid out group-major:
# tabrow(core, slot in group g) = GBASE[g] + core*BROWS[g] + (slot - g start)
GROUPS = [(0, 14), (14, 27), (27, 38), (38, 46), (46, 49)]
BROWS = [(b1 - b0) * 128 for (b0, b1) in GROUPS]
GBASE = [NC * sum(BROWS[:g]) for g in range(len(GROUPS))]


def _tabrow_map():
    tr = np.empty(NP, np.int64)
    for g, (b0, b1) in enumerate(GROUPS):
        s0, s1 = b0 * 128, b1 * 128
        for c in range(NC):
            ids = c * LP + np.arange(s0, s1)
            tr[ids] = GBASE[g] + c * BROWS[g] + np.arange(s1 - s0)
    return tr


TROW = _tabrow_map()
SENT_ID = int(TROW[4 * LP + L])   # core-4 sentinel tab row; rel id >= 0
HEADS = 4
HD = 32
HID = 128
OUT = 64
F0 = 256
NEG = 0.2
CH = 16                  # max slot columns per gather chunk
SENT_EL = -200.0

F32 = mybir.dt.float32
BF16 = mybir.dt.bfloat16
I16 = mybir.dt.int16
AF = mybir.ActivationFunctionType
OP = mybir.AluOpType

# feature interleave map: new col f' = (d, h) <- old col f = h*HD + d
FINT12 = np.array([(f % HEADS) * HD + f // HEADS for f in range(HID)], np.int64)


def _split_multiwaits(nc):
    nsplit = 0
    for bb in nc.main_func.blocks:
        i = 0
        while i < len(bb.instructions):
            ins = bb.instructions[i]
            si = ins.sync_info
            if si is not None and si.on_wait and len(si.on_wait) > 1:
                waits = list(si.on_wait)
                new_insts = []
                for w in waits[:-1]:
                    h = SemaphoreHandle(name=w.ant_name, num=w.id)
                    eng = nc.engines[ins.engine]
                    if w.wait_mode == "sem-ge-imm":
                        wi = eng.wait_ge(h, w.wait_value)
                    elif w.wait_mode == "sem-eq-imm":
                        wi = eng.wait_op(h, w.wait_value, "==")
                    else:
                        raise AssertionError(w.wait_mode)
                    removed = False
                    for b2 in nc.main_func.blocks:
                        if b2.instructions and b2.instructions[-1].name == wi.ins.name:
                            b2.instructions.pop()
                            removed = True
                            break
                    assert removed
                    new_insts.append(wi.ins)
                si.on_wait = [waits[-1]]
                for k, n in enumerate(new_insts):
                    bb.instructions.insert(i + k, n)
                i += len(new_insts)
                nsplit += 1
            i += 1
    return nsplit


def _permute(src, dst):
    """Global in-degree sort, round-robin deal over cores. Every core's
    block-k degree profile matches, minimizing the program-wide per-block
    slot-grid width. Returns perm (old->new), node_order (-1 = sentinel)."""
    deg = np.bincount(dst, minlength=N)
    rank = np.argsort(-deg, kind="stable")
    perm = np.empty(N, np.int64)
    perm[rank] = (np.arange(N) % NC) * LP + np.arange(N) // NC
    node_order = np.full(NP, -1, np.int64)
    node_order[perm] = np.arange(N)
    return perm, node_order


def _cumcount(groups):
    n = len(groups)
    if n == 0:
        return np.zeros(0, np.int64)
    first = np.r_[True, groups[1:] != groups[:-1]]
    idx = np.arange(n)
    return idx - np.repeat(idx[first], np.diff(np.r_[idx[first], n]))


def _preprocess(src, dst):
    src = np.asarray(src, np.int64)
    dst = np.asarray(dst, np.int64)
    perm, node_order = _permute(src, dst)
    nsrc = perm[src]
    ndst = perm[dst]

    cnt = np.bincount(ndst, minlength=NP).reshape(NC, NBLK, 128)
    W = cnt.max(axis=(0, 2))

    def split_w(w):
        out = []
        while w > 0:
            t = min(CH, w)
            out.append(t)
            w -= t
        return out

    # chunk list per block: (width, col_off, idx_off16)
    chunks = []
    Wtot = 0
    S16tot = 0
    col0 = np.zeros(NBLK, np.int64)
    for b in range(NBLK):
        cl = []
        col0[b] = Wtot
        for w in split_w(int(W[b])):
            cl.append((w, Wtot, S16tot))
            Wtot += w
            S16tot += 8 * w
        chunks.append(cl)

    SENT_REL = SENT_ID - CBASE   # >= 0, safe as a chunk-final index
    idx_alls = []
    for c in range(NC):
        m = (ndst // LP) == c
        es = nsrc[m]
        ed = ndst[m] - c * LP
        order = np.argsort(ed, kind="stable")
        es, ed = es[order], ed[order]
        j = _cumcount(ed)

        grid = np.full((128, Wtot), SENT_REL, np.int64)
        blk = ed // 128
        p = ed % 128
        grid[p, col0[blk] + j] = TROW[es] - CBASE

        # the gather ucode drops TRAILING negative indices: the stream is
        # column-major so the last index of a chunk is (p=127, last col).
        # Partition 127's edge columns are interchangeable (same dst) —
        # swap a nonnegative one into each chunk-final slot.
        for b in range(NBLK):
            cend = [coff + w - 1 for (w, coff, _s) in chunks[b]]
            row = grid[127, col0[b]:col0[b] + int(W[b])]
            for ce in cend:
                lc = ce - col0[b]
                if row[lc] < 0:
                    cand = np.where(row >= 0)[0]
                    cand = [x for x in cand if (col0[b] + x) not in cend]
                    assert cand, f"block {b}: no nonneg idx for chunk end"
                    x = cand[0]
                    row[lc], row[x] = row[x], row[lc]
            grid[127, col0[b]:col0[b] + int(W[b])] = row

        pieces = []
        for b in range(NBLK):
            for (w, coff, _s) in chunks[b]:
                flat = grid[:, coff:coff + w].T.reshape(-1)   # i = col*128 + p
                t = flat.reshape(8 * w, 16).T.astype(np.int16)
                tt = np.zeros((128, 8 * w), np.int16)
                for g in range(8):
                    tt[g * 16:(g + 1) * 16] = t
                pieces.append(tt)
        idx_alls.append(np.concatenate(pieces, axis=1))

    meta = dict(chunks=chunks, Wtot=Wtot, S16tot=S16tot,
                node_order=node_order, perm=perm)
    return meta, idx_alls


def _weights_ext(W, al, ar, heads, hd, row_perm, col_int):
    """Extended weight block [Wp | A | B] in bf16. row_perm permutes input
    features (previous layer's interleave); col_int interleaves output cols."""
    W = np.asarray(W, np.float32)
    K = W.shape[0]
    Wr = W.reshape(K, heads, hd)
    A = np.einsum("khd,hd->kh", Wr, np.asarray(al, np.float32))
    B = np.einsum("khd,hd->kh", Wr, np.asarray(ar, np.float32))
    Wp = W[:, col_int] if col_int is not None else W
    We = np.concatenate([Wp, A, B], axis=1)
    if row_perm is not None:
        We = We[row_perm, :]
    pad = (-We.shape[1]) % 4
    if pad:
        We = np.concatenate([We, np.zeros((K, pad), np.float32)], axis=1)
    return We.astype(ml_dtypes.bfloat16)


def _build_program(meta):
    chunks = meta["chunks"]
    S16tot = meta["S16tot"]

    nc = bacc.Bacc("TRN2")

    featT = nc.dram_tensor("featT", [F0, LP], BF16, kind="ExternalInput")
    W1e = nc.dram_tensor("W1e", [F0, 136], BF16, kind="ExternalInput")
    W2e = nc.dram_tensor("W2e", [HID, 136], BF16, kind="ExternalInput")
    W3e = nc.dram_tensor("W3e", [HID, 68], BF16, kind="ExternalInput")
    b1r = nc.dram_tensor("b1r", [128, HID], F32, kind="ExternalInput")
    b2r = nc.dram_tensor("b2r", [128, HID], F32, kind="ExternalInput")
    b3r = nc.dram_tensor("b3r", [128, OUT], F32, kind="ExternalInput")
    ident_in = nc.dram_tensor("identb", [128, 128], BF16, kind="ExternalInput")
    sent_in = nc.dram_tensor("sent", [LP - L, 384], BF16, kind="ExternalInput")
    idx_in = nc.dram_tensor("idx_all", [128, S16tot], I16, kind="ExternalInput")
    out_ext = nc.dram_tensor("out", [LP, OUT], F32, kind="ExternalOutput")

    ROW12, ROW3 = 256, 128
    NG = len(GROUPS)
    rows = [ROW12, ROW12, ROW3]
    tlocs = [[nc.dram_tensor(f"tab_loc{li}_{g}", [BROWS[g], rows[li]], BF16)
              for g in range(NG)] for li in range(3)]
    tab1 = nc.dram_tensor("tab1", [NP, ROW12], BF16, addr_space="Shared")
    tab2 = nc.dram_tensor("tab2", [NP, ROW12], BF16, addr_space="Shared")
    tab3 = nc.dram_tensor("tab3", [NP, ROW3], BF16, addr_space="Shared")

    layers = [
        dict(Fin=F0, Fout=HID, heads=HEADS, W=W1e, ncols=136, row=ROW12,
             tloc=tlocs[0], tfull=tab1, brep=b1r, relu=True),
        dict(Fin=HID, Fout=HID, heads=HEADS, W=W2e, ncols=136, row=ROW12,
             tloc=tlocs[1], tfull=tab2, brep=b2r, relu=True),
        dict(Fin=HID, Fout=OUT, heads=1, W=W3e, ncols=68, row=ROW3,
             tloc=tlocs[2], tfull=tab3, brep=b3r, relu=False),
    ]

    def group_of(cb):
        for g, (b0, b1) in enumerate(GROUPS):
            if b0 <= cb < b1:
                return g
        raise AssertionError(cb)

    with tile.TileContext(nc) as tc:
        with (
            tc.tile_pool(name="persist", bufs=1) as pp,
            tc.tile_pool(name="work", bufs=2) as wp,
            tc.tile_pool(name="mg", bufs=6) as mgp,
            tc.tile_pool(name="wmp", bufs=4) as wmp,
            tc.tile_pool(name="psum", bufs=2, space="PSUM") as psp,
            tc.tile_pool(name="psumA", bufs=3, space="PSUM") as pspA,
            tc.tile_pool(name="psumT", bufs=2, space="PSUM") as pspT,
        ):
            zero_col = pp.tile([128, 1], F32, tag="zero")
            nc.vector.memset(zero_col[:], 0.0)

            # weights/biases first (tiny, gate the first dense block)
            wsbs = []
            biass = []
            for li, lay in enumerate(layers):
                ktiles = lay["Fin"] // 128
                w_t = pp.tile([128, ktiles, lay["ncols"]], BF16, tag=f"wsb{li}")
                for kt in range(ktiles):
                    nc.sync.dma_start(
                        w_t[:, kt, :], lay["W"][kt * 128:(kt + 1) * 128, :])
                wsbs.append(w_t)
                b_t = pp.tile([128, lay["Fout"]], F32, tag=f"bias{li}")
                nc.sync.dma_start(b_t[:], lay["brep"][:, 0:lay["Fout"]])
                biass.append(b_t)

            # featT in column chunks so layer-0 dense starts immediately;
            # idx on the scalar HWDGE ring, in parallel with the sync ring
            xT_a0 = pp.tile([128, LP], BF16, tag="xTa0")
            xT_a1 = pp.tile([128, LP], BF16, tag="xTa1")
            xT_b = pp.tile([128, LP], BF16, tag="xTb")
            FCH = LP // 4
            for k in range(4):
                nc.sync.dma_start(
                    xT_a0[:, k * FCH:(k + 1) * FCH],
                    featT[0:128, k * FCH:(k + 1) * FCH])
                nc.sync.dma_start(
                    xT_a1[:, k * FCH:(k + 1) * FCH],
                    featT[128:256, k * FCH:(k + 1) * FCH])
            idx_sb = pp.tile([128, S16tot], I16, tag="idx")
            nc.scalar.dma_start(idx_sb[:], idx_in[:])
            identb = pp.tile([128, 128], BF16, tag="ident")
            nc.scalar.dma_start(identb[:], ident_in[:])

            # double-buffered er (layer li uses er_ab[li % 2])
            er_ab = [pp.tile([128, NBLK, HEADS], F32, name=f"er{i}", tag=f"er{i}")
                     for i in range(2)]

            gsent = group_of(NBLK - 1)
            soff_sent = L - GROUPS[gsent][0] * 128
            for li, lay in enumerate(layers):
                nc.sync.dma_start(
                    lay["tloc"][gsent][soff_sent:BROWS[gsent], :],
                    sent_in[:, 0:lay["row"]] if li < 2
                    else sent_in[:, 256:256 + lay["row"]])

            def dense_block(li, cb):
                lay = layers[li]
                heads, Fout, ROW = lay["heads"], lay["Fout"], lay["row"]
                ktiles = lay["Fin"] // 128
                xts = [xT_a0, xT_a1][:ktiles] if li == 0 else \
                      ([xT_b] if li == 1 else [xT_a0])
                n0 = cb * 128
                ps = psp.tile([128, lay["ncols"]], F32, tag="dps")
                for kt in range(ktiles):
                    nc.tensor.matmul(
                        ps[:, :], xts[kt][:, n0:n0 + 128], wsbs[li][:, kt, :],
                        start=(kt == 0), stop=(kt == ktiles - 1))
                row_t = wp.tile([128, ROW], BF16, tag="rowt")
                nc.vector.tensor_copy(row_t[:, 0:Fout], ps[:, 0:Fout])
                nc.vector.tensor_copy(
                    row_t[:, Fout:Fout + 2 * heads].bitcast(F32),
                    ps[:, Fout:Fout + heads])
                nc.vector.tensor_copy(
                    er_ab[li % 2][:, cb, 0:heads],
                    ps[:, Fout + heads:Fout + 2 * heads])
                nn = min(128, L - n0)
                g = group_of(cb)
                loff = n0 - GROUPS[g][0] * 128
                nc.sync.dma_start(
                    lay["tloc"][g][loff:loff + nn, :], row_t[0:nn, :])

            def barrier(li, g):
                lay = layers[li]
                cc = nc.gpsimd.collective_compute(
                    "AllGather", OP.bypass,
                    replica_groups=[list(range(NC))],
                    ins=[lay["tloc"][g][:]],
                    outs=[lay["tfull"][GBASE[g]:GBASE[g] + NC * BROWS[g], :]])
                return cc

            def edge_block(li, b, ccs):
                lay = layers[li]
                heads, Fout, ROW = lay["heads"], lay["Fout"], lay["row"]
                elo = Fout
                xt_next = xT_b if li == 0 else (xT_a0 if li == 1 else None)
                TQ = lay["tfull"][CBASE:NP, :]
                bl = chunks[b]
                ncol_b = sum(w for (w, _c, _s) in bl)
                acc = pspA.tile([128, Fout + heads], F32, tag="acc")
                erb = er_ab[li % 2][:, b, 0:heads]
                coli = 0
                for (w, _coff, soff) in bl:
                    mg = mgp.tile([128, CH, ROW], BF16, tag="mg")
                    nidx = 128 * w
                    gi = nc.gpsimd.dma_gather(
                        mg[:, 0:w, 0:ROW], TQ,
                        idx_sb[:, soff:soff + 8 * w],
                        nidx, nidx, ROW, single_packet=False)
                    # the gather reads rows outside its nominal AP (signed
                    # idx), so range-based DRAM tracking can miss group-0:
                    # order explicitly after every group's AllGather.
                    for cc in ccs:
                        bass._add_dep_helper(
                            gi.ins, cc.ins, sync=True, reason="tab ready")
                    elv = mg[:, 0:w, elo:elo + 2 * heads].bitcast(F32)
                    t1 = wp.tile([128, CH, HEADS], F32, tag="t1")
                    nc.vector.tensor_tensor(
                        t1[:, 0:w, 0:heads], elv,
                        erb.unsqueeze(1).broadcast_to([128, w, heads]), OP.add)
                    t2 = wp.tile([128, CH, HEADS], F32, tag="t2")
                    nc.vector.scalar_tensor_tensor(
                        t2[:, 0:w, 0:heads], t1[:, 0:w, 0:heads], NEG,
                        t1[:, 0:w, 0:heads], op0=OP.mult, op1=OP.max)
                    wm = wmp.tile([128, CH, Fout + heads], BF16, tag="wm")
                    nc.scalar.activation(
                        wm[:, 0:w, Fout:Fout + heads],
                        t2[:, 0:w, 0:heads], AF.Exp)
                    nc.vector.tensor_tensor(
                        wm[:, 0:w, 0:Fout].rearrange(
                            "p w (d h) -> p w d h", h=heads),
                        mg[:, 0:w, 0:Fout].rearrange(
                            "p w (d h) -> p w d h", h=heads),
                        wm[:, 0:w, Fout:Fout + heads].unsqueeze(2).broadcast_to(
                            [128, w, Fout // heads, heads]),
                        OP.mult)
                    for ci in range(w):
                        nc.tensor.matmul(
                            acc[:, :], identb[:, :], wm[:, ci, :],
                            start=(coli == 0), stop=(coli == ncol_b - 1))
                        coli += 1
                # normalize + bias (+relu)
                rden = wp.tile([128, heads], F32, tag="rden")
                nc.vector.reciprocal(rden[:], acc[:, Fout:Fout + heads])
                hn = wp.tile([128, Fout], F32, tag="hn")
                nc.vector.tensor_tensor(
                    hn[:].rearrange("p (d h) -> p d h", h=heads),
                    acc[:, 0:Fout].rearrange("p (d h) -> p d h", h=heads),
                    rden.unsqueeze(1).broadcast_to([128, Fout // heads, heads]),
                    OP.mult)
                hb = wp.tile([128, Fout], F32, tag="hb")
                nc.vector.tensor_tensor(hb[:], hn[:], biass[li][:], OP.add)
                if lay["relu"]:
                    ro = wp.tile([128, Fout], BF16, tag="ro")
                    nc.vector.tensor_tensor(
                        ro[:], hb[:],
                        zero_col.broadcast_to([128, Fout]), OP.max)
                    pst = pspT.tile([128, 128], BF16, tag="tps")
                    nc.tensor.transpose(pst[:], ro[:], identb[:])
                    nc.vector.tensor_copy(
                        xt_next[:, b * 128:(b + 1) * 128], pst[:])
                else:
                    nc.sync.dma_start(
                        out_ext[b * 128:(b + 1) * 128, :], hb[:, 0:OUT])

            ends = {b1 - 1: g for g, (_b0, b1) in enumerate(GROUPS)}
            ccs = []
            for cb in range(NBLK):
                dense_block(0, cb)
                if cb in ends:
                    ccs.append(barrier(0, ends[cb]))
            for li in range(3):
                next_ccs = []
                for b in range(NBLK):
                    edge_block(li, b, ccs)
                    if li < 2:
                        dense_block(li + 1, b)
                        if b in ends:
                            next_ccs.append(barrier(li + 1, ends[b]))
                ccs = next_ccs

    _split_multiwaits(nc)
    nc.compile()
    return nc


def _ensure_trace_hook():
    """Dev-only: register the axon NTFF profile hook so trace=True works.

    The agent image lacks antenv.axon_hooks; synthesize it and skip the
    artifact upload (no bucket access here). Never runs in the harness
    (KERNEL_TRACE unset).
    """
    import sys
    import types

    name = "antenv.axon_hooks"
    if name not in sys.modules:
        try:
            from trn_agent_boot.trn_boot import _ntff_profile_via_ctypes
        except ImportError:
            return
        hook = _ntff_profile_via_ctypes("/opt/axon/libaxon_pjrt.so")
        mod = types.ModuleType(name)
        mod._hook = hook
        mod.get_axon_ntff_profile_hook = lambda: mod._hook
        mod.set_axon_ntff_profile_hook = lambda h: setattr(mod, "_hook", h)
        sys.modules[name] = mod
        import antenv

        antenv.axon_hooks = mod
    import concourse.bass_utils as _bu

    _bu.upload_artifacts = lambda tmpdir: tmpdir


_CACHE = {}

LAST_EXEC_NS = None
LAST_TRACE_PATH = None


def kernel(feat, src, dst, W1, al1, ar1, b1, W2, al2, ar2, b2, W3, al3, ar3, b3):
    feat = np.asarray(feat, np.float32)
    key = (int(np.asarray(src[:100]).sum()), int(np.asarray(dst[:100]).sum()))
    if key in _CACHE:
        nc, meta, idx_alls = _CACHE[key]
    else:
        meta, idx_alls = _preprocess(src, dst)
        nc = _build_program(meta)
        _CACHE[key] = (nc, meta, idx_alls)

    node_order = meta["node_order"]

    W1e = _weights_ext(W1, al1, ar1, HEADS, HD, None, FINT12)
    W2e = _weights_ext(W2, al2, ar2, HEADS, HD, FINT12, FINT12)
    W3e = _weights_ext(W3, al3, ar3, 1, OUT, FINT12, None)
    assert W1e.shape[1] == 136 and W3e.shape[1] == 68

    identb = np.eye(128, dtype=ml_dtypes.bfloat16)
    sent = np.zeros((LP - L, 384), np.float32)
    sent_bf = sent.astype(ml_dtypes.bfloat16)
    # L12 sentinel: el fp32 pairs at bf16 cols 128:136; L3: at 64:66
    el12 = np.full((LP - L, 4), SENT_EL, np.float32)
    el3 = np.full((LP - L, 1), SENT_EL, np.float32)
    sent_bf[:, 128:136] = el12.view(np.uint16).view(ml_dtypes.bfloat16)
    sent_bf[:, 256 + 64:256 + 66] = el3.view(np.uint16).view(ml_dtypes.bfloat16)
    b1p = np.asarray(b1, np.float32)[FINT12]
    b2p = np.asarray(b2, np.float32)[FINT12]
    b1r = np.tile(b1p[None, :], (128, 1))
    b2r = np.tile(b2p[None, :], (128, 1))
    b3r = np.tile(np.asarray(b3, np.float32)[None, :], (128, 1))

    in_maps = []
    for c in range(NC):
        nodes = node_order[c * LP:c * LP + L]
        featT_c = np.zeros((F0, LP), ml_dtypes.bfloat16)
        featT_c[:, 0:L] = feat[nodes, :].T
        in_maps.append(dict(
            featT=featT_c, W1e=W1e, W2e=W2e, W3e=W3e,
            b1r=b1r, b2r=b2r, b3r=b3r, identb=identb, sent=sent_bf,
            idx_all=idx_alls[c],
        ))

    trace = os.environ.get("KERNEL_TRACE", "0") == "1"
    tmpdir = None
    if trace:
        _ensure_trace_hook()
        base = os.environ.get("KERNEL_TRACE_DIR")
        if base:
            import tempfile

            os.makedirs(base, exist_ok=True)
            tmpdir = tempfile.mkdtemp(dir=base)
    res = run_bass_kernel_spmd(
        nc, in_maps, list(range(NC)), trace=trace, tmpdir=tmpdir,
    )
    global LAST_EXEC_NS, LAST_TRACE_PATH
    LAST_EXEC_NS = res.exec_time_ns
    it = res.instructions_and_trace
    LAST_TRACE_PATH = it[1] if it else None

    out = np.empty((N, OUT), np.float32)
    for c in range(NC):
        nodes = node_order[c * LP:c * LP + L]
        out[nodes] = res.results[c]["out"][0:L, :]
    return out


# revision 49
# speedup vs baseline: 1.0321x; 1.0321x over previous
"""GAT (3-layer DGL-style) on 8 Trainium2 NeuronCores.

Nodes are globally sorted by in-degree and dealt round-robin to cores/slots
so every core's dst-block degree profile matches, which minimizes the
program-wide slot-grid width (the dominant cost is SWDGE descriptor
generation at ~8 ns/gathered row on the Pool Q7, so padded slots are paid
in wall-clock). Per layer: a bf16 dense matmul produces per-node rows
[h | el | er] with h feature-interleaved (f' = hd*H + h); the rows stream
into per-group local tables whose AllGathers fire as each block group
completes, overlapping the collective with the compute pipeline; then each
core runs the edge phase for its own dst blocks. Gathers use a single pass
over a table base centered at CBASE with *signed* int16 indices (the gather
ucode computes base + idx*stride with sign-extended indices; only trailing
negative indices are dropped, so each chunk's final slot is kept
nonnegative). Per chunk: alpha = exp(lrelu(el+er)) via 2 DVE ops + 1 ACT
exp, the alpha-weighted messages via one broadcast tensor_tensor (legal
because of the feature interleave), and the slot reduction + softmax
denominator on the tensor engine as identity-weight matmuls accumulating in
PSUM. Next layer's dense blocks interleave into the edge loop. Padding
slots point at sentinel table rows (h=0, el=-200) so no masking is needed.
"""

import os

import numpy as np
import ml_dtypes

import concourse.bacc as bacc
import concourse.bass as bass
import concourse.mybir as mybir
from concourse import tile
from concourse.bass_utils import run_bass_kernel_spmd
from bass_rust import SemaphoreHandle

N = 50000
E = 800000
NC = 8
L = 6250                 # real nodes per core
NBLK = 49
LP = NBLK * 128          # padded nodes per core (6272; rows 6250+ are sentinels)
NP = NC * LP             # padded global node count (50176)
CBASE = NP // 2          # gather base row: idx are signed rel ids in [-CBASE, CBASE)

# AllGather groups: block ranges whose tables gather independently so the
# collectives overlap the dense/edge pipeline; last group smallest so the
# final exposed AG is short. Table rows are laid out group-major:
# tabrow(core, slot in group g) = GBASE[g] + core*BROWS[g] + (slot - g start)
GROUPS = [(0, 15), (15, 28), (28, 40), (40, 49)]
BROWS = [(b1 - b0) * 128 for (b0, b1) in GROUPS]
GBASE = [NC * sum(BROWS[:g]) for g in range(len(GROUPS))]


def _tabrow_map():
    tr = np.empty(NP, np.int64)
    for g, (b0, b1) in enumerate(GROUPS):
        s0, s1 = b0 * 128, b1 * 128
        for c in range(NC):
            ids = c * LP + np.arange(s0, s1)
            tr[ids] = GBASE[g] + c * BROWS[g] + np.arange(s1 - s0)
    return tr


TROW = _tabrow_map()
SENT_ID = int(TROW[4 * LP + L])   # core-4 sentinel tab row; rel id >= 0
HEADS = 4
HD = 32
HID = 128
OUT = 64
F0 = 256
NEG = 0.2
CH = 16                  # max slot columns per gather chunk
SENT_EL = -200.0

F32 = mybir.dt.float32
BF16 = mybir.dt.bfloat16
I16 = mybir.dt.int16
AF = mybir.ActivationFunctionType
OP = mybir.AluOpType

# feature interleave map: new col f' = (d, h) <- old col f = h*HD + d
FINT12 = np.array([(f % HEADS) * HD + f // HEADS for f in range(HID)], np.int64)


def _split_multiwaits(nc):
    nsplit = 0
    for bb in nc.main_func.blocks:
        i = 0
        while i < len(bb.instructions):
            ins = bb.instructions[i]
            si = ins.sync_info
            if si is not None and si.on_wait and len(si.on_wait) > 1:
                waits = list(si.on_wait)
                new_insts = []
                for w in waits[:-1]:
                    h = SemaphoreHandle(name=w.ant_name, num=w.id)
                    eng = nc.engines[ins.engine]
                    if w.wait_mode == "sem-ge-imm":
                        wi = eng.wait_ge(h, w.wait_value)
                    elif w.wait_mode == "sem-eq-imm":
                        wi = eng.wait_op(h, w.wait_value, "==")
                    else:
                        raise AssertionError(w.wait_mode)
                    removed = False
                    for b2 in nc.main_func.blocks:
                        if b2.instructions and b2.instructions[-1].name == wi.ins.name:
                            b2.instructions.pop()
                            removed = True
                            break
                    assert removed
                    new_insts.append(wi.ins)
                si.on_wait = [waits[-1]]
                for k, n in enumerate(new_insts):
                    bb.instructions.insert(i + k, n)
                i += len(new_insts)
                nsplit += 1
            i += 1
    return nsplit


def _permute(src, dst):
    """Global in-degree sort, round-robin deal over cores. Every core's
    block-k degree profile matches, minimizing the program-wide per-block
    slot-grid width. Returns perm (old->new), node_order (-1 = sentinel)."""
    deg = np.bincount(dst, minlength=N)
    rank = np.argsort(-deg, kind="stable")
    perm = np.empty(N, np.int64)
    perm[rank] = (np.arange(N) % NC) * LP + np.arange(N) // NC
    node_order = np.full(NP, -1, np.int64)
    node_order[perm] = np.arange(N)
    return perm, node_order


def _cumcount(groups):
    n = len(groups)
    if n == 0:
        return np.zeros(0, np.int64)
    first = np.r_[True, groups[1:] != groups[:-1]]
    idx = np.arange(n)
    return idx - np.repeat(idx[first], np.diff(np.r_[idx[first], n]))


def _preprocess(src, dst):
    src = np.asarray(src, np.int64)
    dst = np.asarray(dst, np.int64)
    perm, node_order = _permute(src, dst)
    nsrc = perm[src]
    ndst = perm[dst]

    cnt = np.bincount(ndst, minlength=NP).reshape(NC, NBLK, 128)
    W = cnt.max(axis=(0, 2))

    def split_w(w):
        out = []
        while w > 0:
            t = min(CH, w)
            out.append(t)
            w -= t
        return out

    # chunk list per block: (width, col_off, idx_off16)
    chunks = []
    Wtot = 0
    S16tot = 0
    col0 = np.zeros(NBLK, np.int64)
    for b in range(NBLK):
        cl = []
        col0[b] = Wtot
        for w in split_w(int(W[b])):
            cl.append((w, Wtot, S16tot))
            Wtot += w
            S16tot += 8 * w
        chunks.append(cl)

    SENT_REL = SENT_ID - CBASE   # >= 0, safe as a chunk-final index
    idx_alls = []
    for c in range(NC):
        m = (ndst // LP) == c
        es = nsrc[m]
        ed = ndst[m] - c * LP
        order = np.argsort(ed, kind="stable")
        es, ed = es[order], ed[order]
        j = _cumcount(ed)

        grid = np.full((128, Wtot), SENT_REL, np.int64)
        blk = ed // 128
        p = ed % 128
        grid[p, col0[blk] + j] = TROW[es] - CBASE

        # the gather ucode drops TRAILING negative indices: the stream is
        # column-major so the last index of a chunk is (p=127, last col).
        # Partition 127's edge columns are interchangeable (same dst) —
        # swap a nonnegative one into each chunk-final slot.
        for b in range(NBLK):
            cend = [coff + w - 1 for (w, coff, _s) in chunks[b]]
            row = grid[127, col0[b]:col0[b] + int(W[b])]
            for ce in cend:
                lc = ce - col0[b]
                if row[lc] < 0:
                    cand = np.where(row >= 0)[0]
                    cand = [x for x in cand if (col0[b] + x) not in cend]
                    assert cand, f"block {b}: no nonneg idx for chunk end"
                    x = cand[0]
                    row[lc], row[x] = row[x], row[lc]
            grid[127, col0[b]:col0[b] + int(W[b])] = row

        pieces = []
        for b in range(NBLK):
            for (w, coff, _s) in chunks[b]:
                flat = grid[:, coff:coff + w].T.reshape(-1)   # i = col*128 + p
                t = flat.reshape(8 * w, 16).T.astype(np.int16)
                tt = np.zeros((128, 8 * w), np.int16)
                for g in range(8):
                    tt[g * 16:(g + 1) * 16] = t
                pieces.append(tt)
        idx_alls.append(np.concatenate(pieces, axis=1))

    meta = dict(chunks=chunks, Wtot=Wtot, S16tot=S16tot,
                node_order=node_order, perm=perm)
    return meta, idx_alls


def _weights_ext(W, al, ar, heads, hd, row_perm, col_int):
    """Extended weight block [Wp | A | B] in bf16. row_perm permutes input
    features (previous layer's interleave); col_int interleaves output cols."""
    W = np.asarray(W, np.float32)
    K = W.shape[0]
    Wr = W.reshape(K, heads, hd)
    A = np.einsum("khd,hd->kh", Wr, np.asarray(al, np.float32))
    B = np.einsum("khd,hd->kh", Wr, np.asarray(ar, np.float32))
    Wp = W[:, col_int] if col_int is not None else W
    We = np.concatenate([Wp, A, B], axis=1)
    if row_perm is not None:
        We = We[row_perm, :]
    pad = (-We.shape[1]) % 4
    if pad:
        We = np.concatenate([We, np.zeros((K, pad), np.float32)], axis=1)
    return We.astype(ml_dtypes.bfloat16)


def _build_program(meta):
    chunks = meta["chunks"]
    S16tot = meta["S16tot"]

    nc = bacc.Bacc("TRN2")

    featT = nc.dram_tensor("featT", [F0, LP], BF16, kind="ExternalInput")
    W1e = nc.dram_tensor("W1e", [F0, 136], BF16, kind="ExternalInput")
    W2e = nc.dram_tensor("W2e", [HID, 136], BF16, kind="ExternalInput")
    W3e = nc.dram_tensor("W3e", [HID, 68], BF16, kind="ExternalInput")
    b1r = nc.dram_tensor("b1r", [128, HID], F32, kind="ExternalInput")
    b2r = nc.dram_tensor("b2r", [128, HID], F32, kind="ExternalInput")
    b3r = nc.dram_tensor("b3r", [128, OUT], F32, kind="ExternalInput")
    ident_in = nc.dram_tensor("identb", [128, 128], BF16, kind="ExternalInput")
    sent_in = nc.dram_tensor("sent", [LP - L, 384], BF16, kind="ExternalInput")
    idx_in = nc.dram_tensor("idx_all", [128, S16tot], I16, kind="ExternalInput")
    out_ext = nc.dram_tensor("out", [LP, OUT], F32, kind="ExternalOutput")

    ROW12, ROW3 = 256, 128
    NG = len(GROUPS)
    rows = [ROW12, ROW12, ROW3]
    tlocs = [[nc.dram_tensor(f"tab_loc{li}_{g}", [BROWS[g], rows[li]], BF16)
              for g in range(NG)] for li in range(3)]
    tab1 = nc.dram_tensor("tab1", [NP, ROW12], BF16, addr_space="Shared")
    tab2 = nc.dram_tensor("tab2", [NP, ROW12], BF16, addr_space="Shared")
    tab3 = nc.dram_tensor("tab3", [NP, ROW3], BF16, addr_space="Shared")

    layers = [
        dict(Fin=F0, Fout=HID, heads=HEADS, W=W1e, ncols=136, row=ROW12,
             tloc=tlocs[0], tfull=tab1, brep=b1r, relu=True),
        dict(Fin=HID, Fout=HID, heads=HEADS, W=W2e, ncols=136, row=ROW12,
             tloc=tlocs[1], tfull=tab2, brep=b2r, relu=True),
        dict(Fin=HID, Fout=OUT, heads=1, W=W3e, ncols=68, row=ROW3,
             tloc=tlocs[2], tfull=tab3, brep=b3r, relu=False),
    ]

    def group_of(cb):
        for g, (b0, b1) in enumerate(GROUPS):
            if b0 <= cb < b1:
                return g
        raise AssertionError(cb)

    with tile.TileContext(nc) as tc:
        with (
            tc.tile_pool(name="persist", bufs=1) as pp,
            tc.tile_pool(name="work", bufs=2) as wp,
            tc.tile_pool(name="mg", bufs=4) as mgp,
            tc.tile_pool(name="wmp", bufs=3) as wmp,
            tc.tile_pool(name="psum", bufs=2, space="PSUM") as psp,
            tc.tile_pool(name="psumA", bufs=3, space="PSUM") as pspA,
            tc.tile_pool(name="psumT", bufs=2, space="PSUM") as pspT,
        ):
            identb = pp.tile([128, 128], BF16, tag="ident")
            nc.sync.dma_start(identb[:], ident_in[:])
            zero_col = pp.tile([128, 1], F32, tag="zero")
            nc.vector.memset(zero_col[:], 0.0)

            # featT in column chunks so layer-0 dense starts immediately;
            # idx on the scalar HWDGE ring, in parallel with the sync ring
            xT_a0 = pp.tile([128, LP], BF16, tag="xTa0")
            xT_a1 = pp.tile([128, LP], BF16, tag="xTa1")
            xT_b = pp.tile([128, LP], BF16, tag="xTb")
            FCH = LP // 4
            for k in range(4):
                nc.sync.dma_start(
                    xT_a0[:, k * FCH:(k + 1) * FCH],
                    featT[0:128, k * FCH:(k + 1) * FCH])
                nc.sync.dma_start(
                    xT_a1[:, k * FCH:(k + 1) * FCH],
                    featT[128:256, k * FCH:(k + 1) * FCH])
            idx_sb = pp.tile([128, S16tot], I16, tag="idx")
            nc.scalar.dma_start(idx_sb[:], idx_in[:])

            # all weights/biases resident
            wsbs = []
            biass = []
            for li, lay in enumerate(layers):
                ktiles = lay["Fin"] // 128
                w_t = pp.tile([128, ktiles, lay["ncols"]], BF16, tag=f"wsb{li}")
                for kt in range(ktiles):
                    nc.sync.dma_start(
                        w_t[:, kt, :], lay["W"][kt * 128:(kt + 1) * 128, :])
                wsbs.append(w_t)
                b_t = pp.tile([128, lay["Fout"]], F32, tag=f"bias{li}")
                nc.sync.dma_start(b_t[:], lay["brep"][:, 0:lay["Fout"]])
                biass.append(b_t)

            # double-buffered er (layer li uses er_ab[li % 2])
            er_ab = [pp.tile([128, NBLK, HEADS], F32, name=f"er{i}", tag=f"er{i}")
                     for i in range(2)]

            gsent = group_of(NBLK - 1)
            soff_sent = L - GROUPS[gsent][0] * 128
            for li, lay in enumerate(layers):
                nc.sync.dma_start(
                    lay["tloc"][gsent][soff_sent:BROWS[gsent], :],
                    sent_in[:, 0:lay["row"]] if li < 2
                    else sent_in[:, 256:256 + lay["row"]])

            def dense_block(li, cb):
                lay = layers[li]
                heads, Fout, ROW = lay["heads"], lay["Fout"], lay["row"]
                ktiles = lay["Fin"] // 128
                xts = [xT_a0, xT_a1][:ktiles] if li == 0 else \
                      ([xT_b] if li == 1 else [xT_a0])
                n0 = cb * 128
                ps = psp.tile([128, lay["ncols"]], F32, tag="dps")
                for kt in range(ktiles):
                    nc.tensor.matmul(
                        ps[:, :], xts[kt][:, n0:n0 + 128], wsbs[li][:, kt, :],
                        start=(kt == 0), stop=(kt == ktiles - 1))
                row_t = wp.tile([128, ROW], BF16, tag="rowt")
                nc.vector.tensor_copy(row_t[:, 0:Fout], ps[:, 0:Fout])
                nc.vector.tensor_copy(
                    row_t[:, Fout:Fout + 2 * heads].bitcast(F32),
                    ps[:, Fout:Fout + heads])
                nc.vector.tensor_copy(
                    er_ab[li % 2][:, cb, 0:heads],
                    ps[:, Fout + heads:Fout + 2 * heads])
                nn = min(128, L - n0)
                g = group_of(cb)
                loff = n0 - GROUPS[g][0] * 128
                nc.sync.dma_start(
                    lay["tloc"][g][loff:loff + nn, :], row_t[0:nn, :])

            def barrier(li, g):
                lay = layers[li]
                cc = nc.gpsimd.collective_compute(
                    "AllGather", OP.bypass,
                    replica_groups=[list(range(NC))],
                    ins=[lay["tloc"][g][:]],
                    outs=[lay["tfull"][GBASE[g]:GBASE[g] + NC * BROWS[g], :]])
                return cc

            def edge_block(li, b, ccs):
                lay = layers[li]
                heads, Fout, ROW = lay["heads"], lay["Fout"], lay["row"]
                elo = Fout
                xt_next = xT_b if li == 0 else (xT_a0 if li == 1 else None)
                TQ = lay["tfull"][CBASE:NP, :]
                bl = chunks[b]
                ncol_b = sum(w for (w, _c, _s) in bl)
                acc = pspA.tile([128, Fout + heads], F32, tag="acc")
                erb = er_ab[li % 2][:, b, 0:heads]
                coli = 0
                for (w, _coff, soff) in bl:
                    mg = mgp.tile([128, CH, ROW], BF16, tag="mg")
                    nidx = 128 * w
                    gi = nc.gpsimd.dma_gather(
                        mg[:, 0:w, 0:ROW], TQ,
                        idx_sb[:, soff:soff + 8 * w],
                        nidx, nidx, ROW, single_packet=False)
                    # the gather reads rows outside its nominal AP (signed
                    # idx), so range-based DRAM tracking can miss group-0:
                    # order explicitly after every group's AllGather.
                    for cc in ccs:
                        bass._add_dep_helper(
                            gi.ins, cc.ins, sync=True, reason="tab ready")
                    elv = mg[:, 0:w, elo:elo + 2 * heads].bitcast(F32)
                    t1 = wp.tile([128, CH, HEADS], F32, tag="t1")
                    nc.vector.tensor_tensor(
                        t1[:, 0:w, 0:heads], elv,
                        erb.unsqueeze(1).broadcast_to([128, w, heads]), OP.add)
                    t2 = wp.tile([128, CH, HEADS], F32, tag="t2")
                    nc.vector.scalar_tensor_tensor(
                        t2[:, 0:w, 0:heads], t1[:, 0:w, 0:heads], NEG,
                        t1[:, 0:w, 0:heads], op0=OP.mult, op1=OP.max)
                    wm = wmp.tile([128, CH, Fout + heads], BF16, tag="wm")
                    nc.scalar.activation(
                        wm[:, 0:w, Fout:Fout + heads],
                        t2[:, 0:w, 0:heads], AF.Exp)
                    nc.vector.tensor_tensor(
                        wm[:, 0:w, 0:Fout].rearrange(
                            "p w (d h) -> p w d h", h=heads),
                        mg[:, 0:w, 0:Fout].rearrange(
                            "p w (d h) -> p w d h", h=heads),
                        wm[:, 0:w, Fout:Fout + heads].unsqueeze(2).broadcast_to(
                            [128, w, Fout // heads, heads]),
                        OP.mult)
                    for ci in range(w):
                        nc.tensor.matmul(
                            acc[:, :], identb[:, :], wm[:, ci, :],
                            start=(coli == 0), stop=(coli == ncol_b - 1))
                        coli += 1
                # normalize + bias (+relu)
                rden = wp.tile([128, heads], F32, tag="rden")
                nc.vector.reciprocal(rden[:], acc[:, Fout:Fout + heads])
                hn = wp.tile([128, Fout], F32, tag="hn")
                nc.vector.tensor_tensor(
                    hn[:].rearrange("p (d h) -> p d h", h=heads),
                    acc[:, 0:Fout].rearrange("p (d h) -> p d h", h=heads),
                    rden.unsqueeze(1).broadcast_to([128, Fout // heads, heads]),
                    OP.mult)
                hb = wp.tile([128, Fout], F32, tag="hb")
                nc.vector.tensor_tensor(hb[:], hn[:], biass[li][:], OP.add)
                if lay["relu"]:
                    ro = wp.tile([128, Fout], BF16, tag="ro")
                    nc.vector.tensor_tensor(
                        ro[:], hb[:],
                        zero_col.broadcast_to([128, Fout]), OP.max)
                    pst = pspT.tile([128, 128], BF16, tag="tps")
                    nc.tensor.transpose(pst[:], ro[:], identb[:])
                    nc.vector.tensor_copy(
                        xt_next[:, b * 128:(b + 1) * 128], pst[:])
                else:
                    nc.sync.dma_start(
                        out_ext[b * 128:(b + 1) * 128, :], hb[:, 0:OUT])

            ends = {b1 - 1: g for g, (_b0, b1) in enumerate(GROUPS)}
            ccs = []
            for cb in range(NBLK):
                dense_block(0, cb)
                if cb in ends:
                    ccs.append(barrier(0, ends[cb]))
            for li in range(3):
                next_ccs = []
                for b in range(NBLK):
                    edge_block(li, b, ccs)
                    if li < 2:
                        dense_block(li + 1, b)
                        if b in ends:
                            next_ccs.append(barrier(li + 1, ends[b]))
                ccs = next_ccs

    _split_multiwaits(nc)
    nc.compile()
    return nc


def _ensure_trace_hook():
    """Dev-only: register the axon NTFF profile hook so trace=True works.

    The agent image lacks antenv.axon_hooks; synthesize it and skip the
    artifact upload (no bucket access here). Never runs in the harness
    (KERNEL_TRACE unset).
    """
    import sys
    import types

    name = "antenv.axon_hooks"
    if name not in sys.modules:
        try:
            from trn_agent_boot.trn_boot import _ntff_profile_via_ctypes
        except ImportError:
            return
        hook = _ntff_profile_via_ctypes("/opt/axon/libaxon_pjrt.so")
        mod = types.ModuleType(name)
        mod._hook = hook
        mod.get_axon_ntff_profile_hook = lambda: mod._hook
        mod.set_axon_ntff_profile_hook = lambda h: setattr(mod, "_hook", h)
        sys.modules[name] = mod
        import antenv

        antenv.axon_hooks = mod
    import concourse.bass_utils as _bu

    _bu.upload_artifacts = lambda tmpdir: tmpdir


_CACHE = {}

LAST_EXEC_NS = None
LAST_TRACE_PATH = None


def kernel(feat, src, dst, W1, al1, ar1, b1, W2, al2, ar2, b2, W3, al3, ar3, b3):
    feat = np.asarray(feat, np.float32)
    key = (int(np.asarray(src[:100]).sum()), int(np.asarray(dst[:100]).sum()))
    if key in _CACHE:
        nc, meta, idx_alls = _CACHE[key]
    else:
        meta, idx_alls = _preprocess(src, dst)
        nc = _build_program(meta)
        _CACHE[key] = (nc, meta, idx_alls)

    node_order = meta["node_order"]

    W1e = _weights_ext(W1, al1, ar1, HEADS, HD, None, FINT12)
    W2e = _weights_ext(W2, al2, ar2, HEADS, HD, FINT12, FINT12)
    W3e = _weights_ext(W3, al3, ar3, 1, OUT, FINT12, None)
    assert W1e.shape[1] == 136 and W3e.shape[1] == 68

    identb = np.eye(128, dtype=ml_dtypes.bfloat16)
    sent = np.zeros((LP - L, 384), np.float32)
    sent_bf = sent.astype(ml_dtypes.bfloat16)
    # L12 sentinel: el fp32 pairs at bf16 cols 128:136; L3: at 64:66
    el12 = np.full((LP - L, 4), SENT_EL, np.float32)
    el3 = np.full((LP - L, 1), SENT_EL, np.float32)
    sent_bf[:, 128:136] = el12.view(np.uint16).view(ml_dtypes.bfloat16)
    sent_bf[:, 256 + 64:256 + 66] = el3.view(np.uint16).view(ml_dtypes.bfloat16)
    b1p = np.asarray(b1, np.float32)[FINT12]
    b2p = np.asarray(b2, np.float32)[FINT12]
    b1r = np.tile(b1p[None, :], (128, 1))
    b2r = np.tile(b2p[None, :], (128, 1))
    b3r = np.tile(np.asarray(b3, np.float32)[None, :], (128, 1))

    in_maps = []
    for c in range(NC):
        nodes = node_order[c * LP:c * LP + L]
        featT_c = np.zeros((F0, LP), ml_dtypes.bfloat16)
        featT_c[:, 0:L] = feat[nodes, :].T
        in_maps.append(dict(
            featT=featT_c, W1e=W1e, W2e=W2e, W3e=W3e,
            b1r=b1r, b2r=b2r, b3r=b3r, identb=identb, sent=sent_bf,
            idx_all=idx_alls[c],
        ))

    trace = os.environ.get("KERNEL_TRACE", "0") == "1"
    tmpdir = None
    if trace:
        _ensure_trace_hook()
        base = os.environ.get("KERNEL_TRACE_DIR")
        if base:
            import tempfile

            os.makedirs(base, exist_ok=True)
            tmpdir = tempfile.mkdtemp(dir=base)
    res = run_bass_kernel_spmd(
        nc, in_maps, list(range(NC)), trace=trace, tmpdir=tmpdir,
    )
    global LAST_EXEC_NS, LAST_TRACE_PATH
    LAST_EXEC_NS = res.exec_time_ns
    it = res.instructions_and_trace
    LAST_TRACE_PATH = it[1] if it else None

    out = np.empty((N, OUT), np.float32)
    for c in range(NC):
        nodes = node_order[c * LP:c * LP + L]
        out[nodes] = res.results[c]["out"][0:L, :]
    return out
